# revision 17
# baseline (speedup 1.0000x reference)
"""Dense transformer block (attention + post-LN MLP) on 8 trn2 NeuronCores.

Context-parallel sharding: core c handles batch c//4 and token blocks
j*256, (7-j)*256 of that batch (j = c%4) -> causally balanced 512
tokens/core. Weights replicated (bf16).

v2 structure:
 - Host ships every tensor in exact SBUF layout (dense per-partition
   lines, one DMA each); the big phase-1 loads are issued first so the
   PE starts early.
 - K is projected FIRST and shipped as its own AllGather; V follows as
   two half-AllGathers (heads 0-7, 8-15); Q is projected while the
   rings run. Remote attention unblocks progressively.
 - Attention is two-pass: ALL head-pairs' local chunks run during the
   rings (partial osum evacuation), then the remote pass combines into
   osum. Deferred softmax normalization in two halves (hp 0-3 while
   hp 4-7 still compute) using reciprocal_approx_fast + GPSIMD
   partition_broadcast.
 - LayerNorms: bf16 cast + square fused into the producer phase,
   column-sum matmuls back-to-back, g-folded 2-op normalize, prewarmed
   Sqrt/Gelu tables. Residuals use bf16 x/n (tolerance 2e-2).
"""

import numpy as np
import ml_dtypes

BF16 = ml_dtypes.bfloat16

N_CORES = 8
B, S, D = 2, 2048, 1024
H, HD = 16, 64
F = 4 * D
TOK = 512            # tokens per core
P = 128
NHP = H // 2         # 8 head pairs
MASK_NEG = -80000.0  # -> -79872 in bf16; /8 => exp underflows to exactly 0
LN_EPS = 1e-5

VW = 66              # per-head V row width: 64 v + ones col + pad
KT_ELEMS = P * 8 * TOK           # K payload per core
VH_ELEMS = P * 4 * (8 * VW)      # V half payload per core

_CACHE = {}


def _q_blocks(j):
    """Global 128-blocks of this core's query tokens, ascending."""
    return [2 * j, 2 * j + 1, 14 - 2 * j, 15 - 2 * j]


def _remote_plan(j):
    """Remote chunks in assumed ring-arrival order (ranks j-1, j-2, j-3),
    each rank's valid chunks ascending.  Returns (mq, [(kc, slot, lt)],
    rank_of_slot)."""
    gq = _q_blocks(j)
    last_kc = gq[-1]
    plan = []
    rank_of_slot = []
    for d in range(1, 4):
        r = (j - d) % 4
        rb = _q_blocks(r)
        got = False
        for lt, kc in enumerate(rb):
            if kc <= last_kc and kc not in gq:
                plan.append((kc, len(rank_of_slot), lt))
                got = True
        if got:
            rank_of_slot.append(r)
    mq = 256 if j == 0 else 0
    return mq, plan, rank_of_slot


def _build():
    import concourse.bass as bass
    import concourse.mybir as mybir
    import concourse.tile as tile
    from concourse import bacc

    dt = mybir.dt
    AF = mybir.ActivationFunctionType
    OP = mybir.AluOpType

    nc = bacc.Bacc(
        "TRN2",
        target_bir_lowering=False,
        debug=False,
        enable_asserts=True,
        num_devices=N_CORES,
    )

    def din(name, shape, dty):
        return nc.dram_tensor(name, shape, dty, kind="ExternalInput").ap()

    # host ships everything in SBUF-exact layout
    x_bf = din("x_bf", [P, 8, TOK], dt.bfloat16)
    wq_d = din("wq_d", [P, 8, D], dt.bfloat16)     # Q columns of w_attn
    wk_d = din("wk_d", [P, 8, D], dt.bfloat16)     # K columns
    wv_d = din("wv_d", [P, 8, D], dt.bfloat16)     # V columns
    wo_d = din("wo_d", [P, 8, D], dt.bfloat16)
    wfc_d = din("wfc_d", [P, 8, F], dt.bfloat16)
    wpr_d = din("wpr_d", [P, 32, D], dt.bfloat16)
    # bias pack cols: bqk 16 | bo 8 | bfc 32 | bpr 8 | g1 8 | b1 8 | g2 8 | b2 8
    bias_d = din("bias_d", [P, 96], dt.float32)
    gcol_d = din("gcol_d", [1, 2 * D], dt.float32)  # g1 | g2 by feature
    im_d = din("im_d", [P, 2, P], dt.bfloat16)      # ident | maskm
    sel8_d = din("sel8", [8, 8 * 64], dt.bfloat16)  # one-hot row selectors
    jidx_d = din("jidx", [1, 1], dt.uint32)
    out_d = nc.dram_tensor("out", [D, TOK], dt.float32, kind="ExternalOutput").ap()

    with tile.TileContext(nc) as tc:
        from contextlib import ExitStack

        ctx = ExitStack()
        with ctx:
            c_pool = ctx.enter_context(tc.tile_pool(name="consts", bufs=1))
            dram = ctx.enter_context(tc.tile_pool(name="dram", bufs=1, space="DRAM"))

            # ---- long-lived pools (stack: pR > kvp > pA > gat > transient) ----
            pR_cm = tc.tile_pool(name="pR", bufs=1)
            pR = pR_cm.__enter__()
            kvp_cm = tc.tile_pool(name="kv_own", bufs=1)
            kvp = kvp_cm.__enter__()
            pA_cm = tc.tile_pool(name="pA", bufs=1)
            pA = pA_cm.__enter__()
            gat_cm = tc.tile_pool(name="gat", bufs=1)
            gat = gat_cm.__enter__()

            # ---- phase-1 critical loads FIRST (sync engine) ----
            xb = pA.tile([P, 8, TOK], dt.bfloat16, tag="xb")
            nc.sync.dma_start(xb[:], x_bf[:])

            xwk_cm = tc.tile_pool(name="xwk", bufs=2)
            xwk = xwk_cm.__enter__()
            wk = xwk.tile([P, 8, D], dt.bfloat16, tag="w8", name="wk")
            nc.sync.dma_start(wk[:], wk_d[:])
            wv = xwk.tile([P, 8, D], dt.bfloat16, tag="w8", name="wv")
            nc.sync.dma_start(wv[:], wv_d[:])

            # ---- per-core j register (for Switch) ----
            jreg = nc.alloc_registers(
                "jreg",
                [mybir.EngineType.PE, mybir.EngineType.Activation,
                 mybir.EngineType.DVE, mybir.EngineType.SP,
                 mybir.EngineType.Pool],
            )
            nc.regs_load(jreg, jidx_d[0:1, 0:1])
            jval = nc.snap(jreg, donate=True, min_val=0, max_val=3)

            # ---- small consts (gpsimd queue keeps sync free) ----
            bias_sb = c_pool.tile([P, 96], dt.float32, tag="bias")
            nc.gpsimd.dma_start(bias_sb[:], bias_d[:])
            im_sb = c_pool.tile([P, 2, P], dt.bfloat16, tag="im")
            nc.gpsimd.dma_start(im_sb[:], im_d[:])
            ones128_bf = c_pool.tile([P, 1], dt.bfloat16, tag="ones128")
            nc.vector.memset(ones128_bf[:], 1.0)
            sel8 = c_pool.tile([8, 8, 64], dt.bfloat16, tag="sel8")
            nc.gpsimd.dma_start(
                sel8[:], sel8_d.rearrange("p (c f) -> p c f", f=64))
            sq_warm = c_pool.tile([1, 1], dt.float32, tag="sqwarm")
            nc.vector.memset(sq_warm[:], 1.0)

            bqk_sb = bias_sb[:, 0:16]
            bo_sb = bias_sb[:, 16:24]
            bfc_sb = bias_sb[:, 24:56]
            bpr_sb = bias_sb[:, 56:64]
            g1_sb = bias_sb[:, 64:72]
            b1_sb = bias_sb[:, 72:80]
            g2_sb = bias_sb[:, 80:88]
            b2_sb = bias_sb[:, 88:96]
            ident = im_sb[:, 0, :]
            maskm = im_sb[:, 1, :]

            # ---- AllGather buffers: K, V-half1, V-half2 ----
            agk_in = dram.tile([KT_ELEMS], dt.bfloat16, tag="agki", name="agki")
            agk_out = dram.tile([4, KT_ELEMS], dt.bfloat16, tag="agko", name="agko")
            agv1_in = dram.tile([VH_ELEMS], dt.bfloat16, tag="agv1i", name="agv1i")
            agv1_out = dram.tile([4, VH_ELEMS], dt.bfloat16, tag="agv1o", name="agv1o")
            agv2_in = dram.tile([VH_ELEMS], dt.bfloat16, tag="agv2i", name="agv2i")
            agv2_out = dram.tile([4, VH_ELEMS], dt.bfloat16, tag="agv2o", name="agv2o")
            GROUPS = [[0, 1, 2, 3], [4, 5, 6, 7]]

            # ---- long-lived SBUF tiles ----
            r1 = pR.tile([P, 8, TOK], dt.float32, tag="r1")

            v_own = kvp.tile([P, 4, 16 * VW], dt.bfloat16, tag="v_own")
            kT_own = kvp.tile([P, 8, TOK], dt.bfloat16, tag="kT_own")
            qT = kvp.tile([P, 8, TOK], dt.bfloat16, tag="qT")

            o_cat = [pA.tile([P, TOK], dt.bfloat16, tag=f"o_cat{i}", name=f"o_cat{i}")
                     for i in range(NHP)]
            wo_sb = pA.tile([P, 8, D], dt.bfloat16, tag="wo")

            kT_s = [gat.tile([P, 8, TOK], dt.bfloat16, tag=f"kTs{s}", name=f"kTs{s}")
                    for s in range(3)]
            v_s = [gat.tile([P, 4, 16 * VW], dt.bfloat16, tag=f"vs{s}",
                            name=f"vs{s}") for s in range(3)]
            osum = [gat.tile([65, 2, TOK], dt.bfloat16, tag=f"osum{i}",
                             name=f"osum{i}") for i in range(NHP)]
            dcath = [gat.tile([4, 2, TOK], dt.bfloat16, tag=f"dcat{i}",
                              name=f"dcat{i}") for i in range(2)]
            dcat_f = gat.tile([4, 2, TOK], dt.float32, tag="dcat_f")
            rcat_f = gat.tile([4, 2, TOK], dt.float32, tag="rcat_f")
            rcath = [gat.tile([4, 2, TOK], dt.bfloat16, tag=f"rcat{i}",
                              name=f"rcat{i}") for i in range(2)]

            # ============ phase 1: K -> ship, V -> ship x2, Q ============
            with tc.tile_pool(name="qkv_ps", bufs=4, space="PSUM") as qkv_ps:

                def qk_quarter(w_sb, qi, dest, kbias):
                    for fo in range(4):
                        fchunk = 4 * qi + fo
                        ps = qkv_ps.tile([P, TOK], dt.float32, tag="qkvps")
                        for c8 in range(8):
                            nc.tensor.matmul(
                                ps[:],
                                lhsT=w_sb[:, c8, qi * TOK + fo * P:
                                          qi * TOK + (fo + 1) * P],
                                rhs=xb[:, c8, :],
                                start=(c8 == 0),
                                stop=(c8 == 7),
                            )
                        bcol = fchunk + (8 if kbias else 0)
                        nc.vector.tensor_scalar(
                            dest[:, fchunk, :], ps[:],
                            bqk_sb[:, bcol:bcol + 1], None, OP.add,
                        )

                # K first
                qk_quarter(wk, 0, kT_own, True)
                qk_quarter(wk, 1, kT_own, True)
                nc.sync.dma_start(
                    agk_in.rearrange("(c p t) -> p c t", c=8, p=P), kT_own[:]
                )
                nc.gpsimd.collective_compute(
                    "AllGather", mybir.AluOpType.bypass,
                    replica_groups=GROUPS,
                    ins=[agk_in.opt()], outs=[agk_out.opt()],
                )
                # wq rotates into wk's slot once the K quarters are done
                wq = xwk.tile([P, 8, D], dt.bfloat16, tag="w8", name="wq")
                nc.sync.dma_start(wq[:], wq_d[:])

                # V next
                v4 = v_own.rearrange("p c (h w) -> p c h w", w=VW)
                nc.vector.memset(v4[:, :, :, 64:66], 0.0)
                nc.vector.memset(v4[:, :, :, 64:65], 1.0)
                for t4 in range(4):
                    for vc in range(2):
                        ps = qkv_ps.tile([P, TOK], dt.float32, tag="qkvps")
                        for c8 in range(8):
                            nc.tensor.matmul(
                                ps[:],
                                lhsT=xb[:, c8, t4 * P:(t4 + 1) * P],
                                rhs=wv[:, c8, vc * TOK:(vc + 1) * TOK],
                                start=(c8 == 0),
                                stop=(c8 == 7),
                            )
                        dest = v4[:, t4, vc * 8:(vc + 1) * 8, 0:64]
                        nc.vector.tensor_copy(
                            dest, ps[:].rearrange("p (h w) -> p h w", w=64)
                        )
                nc.sync.dma_start(
                    agv1_in.rearrange("(c p f) -> p c f", c=4, p=P),
                    v_own[:, :, 0:8 * VW],
                )
                nc.gpsimd.collective_compute(
                    "AllGather", mybir.AluOpType.bypass,
                    replica_groups=GROUPS,
                    ins=[agv1_in.opt()], outs=[agv1_out.opt()],
                )
                nc.sync.dma_start(
                    agv2_in.rearrange("(c p f) -> p c f", c=4, p=P),
                    v_own[:, :, 8 * VW:16 * VW],
                )
                nc.gpsimd.collective_compute(
                    "AllGather", mybir.AluOpType.bypass,
                    replica_groups=GROUPS,
                    ins=[agv2_in.opt()], outs=[agv2_out.opt()],
                )

                # Q last (overlaps the rings)
                qk_quarter(wq, 0, qT, False)
                qk_quarter(wq, 1, qT, False)

            xwk_cm.__exit__(None, None, None)

            # prefetch w_o for the post-attention projection
            nc.sync.dma_start(wo_sb[:], wo_d[:])

            # ============ attention ============
            with (
                tc.tile_pool(name="att_s", bufs=2, space="PSUM") as s_ps_pool,
                tc.tile_pool(name="att_o", bufs=2, space="PSUM") as o_ps_pool,
                tc.tile_pool(name="att_tmp", bufs=3) as atmp,
                tc.tile_pool(name="att_nrm", bufs=2) as anrm,
            ):
                def attn_qkav(hp, kc, gq, oT, first, last, ksrc, vsrc):
                    vi = [i for i in range(4) if gq[i] >= kc]
                    qs = P * vi[0]
                    diag = gq[vi[0]] == kc
                    sps = s_ps_pool.tile([P, 2, TOK], dt.float32, tag="sps")
                    for h2 in range(2):
                        lo, hi = 64 * h2, 64 * (h2 + 1)
                        nc.tensor.matmul(
                            sps[:, h2, qs:],
                            lhsT=ksrc[lo:hi, :],
                            rhs=qT[lo:hi, hp, qs:],
                            start=True,
                            stop=not diag,
                            tile_position=(64 * h2, 0),
                        )
                        if diag:
                            nc.tensor.matmul(
                                sps[:, h2, qs:qs + P],
                                lhsT=ident[:],
                                rhs=maskm[:],
                                start=False,
                                stop=True,
                            )
                    aT = atmp.tile([P, 2, TOK], dt.bfloat16, tag="aT")
                    nc.scalar.activation(
                        aT[:, :, qs:], sps[:, :, qs:], AF.Exp, scale=0.125
                    )
                    for h2 in range(2):
                        h = 2 * hp + h2
                        nc.tensor.matmul(
                            oT[:, h2, qs:],
                            lhsT=vsrc[:, h * VW:h * VW + 65],
                            rhs=aT[:, h2, qs:],
                            start=first,
                            stop=last,
                        )

                def norm_half(h0):
                    """Deferred softmax normalization for hps h0..h0+3."""
                    half = h0 // 4
                    rc = rcath[half]
                    nc.vector.tensor_copy(dcat_f[:], dcath[half][:])
                    nc.vector.reciprocal_approx_fast(rcat_f[:], dcat_f[:])
                    nc.vector.tensor_copy(rc[:], rcat_f[:])
                    for hp in range(h0, h0 + 4):
                        rep = s_ps_pool.tile([P, 2, TOK], dt.float32, tag="sps")
                        for h2 in range(2):
                            nc.tensor.matmul(
                                rep[0:64, h2, :],
                                lhsT=sel8[0:4, hp - h0, :],
                                rhs=rc[:, h2, :],
                                start=True, stop=True,
                            )
                        ot = anrm.tile([64, TOK], dt.bfloat16, tag="otmp")
                        nc.vector.tensor_tensor(
                            o_cat[hp][0:64, :], osum[hp][0:64, 0, :],
                            rep[0:64, 0, :], OP.mult,
                        )
                        nc.vector.tensor_tensor(
                            ot[:], osum[hp][0:64, 1, :], rep[0:64, 1, :], OP.mult,
                        )
                        nc.sync.dma_start(o_cat[hp][64:128, :], ot[:])

                def attn_arm(j):
                    gq = _q_blocks(j)
                    mq, plan, rank_of_slot = _remote_plan(j)
                    for s, r in enumerate(rank_of_slot):
                        nc.sync.dma_start(
                            kT_s[s][:],
                            agk_out[r].rearrange("(c p t) -> p c t", c=8, p=P),
                        )
                        nc.sync.dma_start(
                            v_s[s][:, :, 0:8 * VW],
                            agv1_out[r].rearrange("(c p f) -> p c f", c=4, p=P),
                        )
                        nc.sync.dma_start(
                            v_s[s][:, :, 8 * VW:16 * VW],
                            agv2_out[r].rearrange("(c p f) -> p c f", c=4, p=P),
                        )
                    # pass 1: local chunks for ALL hps (overlaps the rings)
                    for hp in range(NHP):
                        oT = o_ps_pool.tile([65, 2, TOK], dt.float32, tag="oT")
                        for ki, kc in enumerate(gq):
                            li = gq.index(kc)
                            attn_qkav(hp, kc, gq, oT, ki == 0, ki == 3,
                                      kT_own[:, hp, li * P:(li + 1) * P],
                                      v_own[:, li, :])
                        nc.vector.tensor_copy(osum[hp][:], oT[:])
                    # pass 2: remote chunks in ring-arrival order, combine
                    for hp in range(NHP):
                        oT = o_ps_pool.tile([65, 2, TOK], dt.float32, tag="oT")
                        for ki, (kc, s, lt) in enumerate(plan):
                            attn_qkav(hp, kc, gq, oT, ki == 0,
                                      ki == len(plan) - 1,
                                      kT_s[s][:, hp, lt * P:(lt + 1) * P],
                                      v_s[s][:, lt, :])
                        nc.vector.tensor_tensor(
                            osum[hp][:, :, mq:], osum[hp][:, :, mq:],
                            oT[:, :, mq:], OP.add,
                        )
                        nc.sync.dma_start(
                            dcath[hp // 4][hp % 4:hp % 4 + 1, :, :],
                            osum[hp][64:65, :, :])
                        if hp == 3:
                            norm_half(0)
                    norm_half(4)

                for case in tc.Switch(jval, 4):
                    attn_arm(case)

            gat_cm.__exit__(None, None, None)

            # prewarm Sqrt table for LN1 (ACT idle during w_o)
            nc.scalar.activation(sq_warm[:], sq_warm[:], AF.Sqrt)

            # ============ phase 4: w_o + residual -> r1 (+ LN1 prep) ============
            with tc.tile_pool(name="wo_ps", bufs=4, space="PSUM") as wo_ps:
                for oc in range(8):
                    ps = wo_ps.tile([P, TOK], dt.float32, tag="wops")
                    for hp in range(8):
                        nc.tensor.matmul(
                            ps[:],
                            lhsT=wo_sb[:, hp, oc * P:(oc + 1) * P],
                            rhs=o_cat[hp][:],
                            start=(hp == 0),
                            stop=(hp == 7),
                        )
                    nc.vector.scalar_tensor_tensor(
                        r1[:, oc, :], ps[:], bo_sb[:, oc:oc + 1], xb[:, oc, :],
                        op0=OP.add, op1=OP.add,
                    )

            pA_cm.__exit__(None, None, None)
            kvp_cm.__exit__(None, None, None)

            # ============ layernorm (feature-major, partition reduce) ============
            def layernorm(src, src_bf, sq, dst, g_sb, b_sb, gcol_off, tag):
                """dst[:, c8, :] = LN(src)[c8] * g + b.  src fp32 [P,8,TOK];
                src_bf/sq bf16 precomputed.  dst may alias src (per-c8 safe)."""
                with (
                    tc.tile_pool(name=f"ln_{tag}", bufs=1) as lnp,
                    tc.tile_pool(name=f"lnps_{tag}", bufs=1, space="PSUM") as lnps,
                    tc.tile_pool(name=f"lnsh_{tag}", bufs=2, space="PSUM") as lnsh,
                ):
                    s12 = lnps.tile([1, 2, TOK], dt.float32, tag=f"s12_{tag}")
                    for c8 in range(8):
                        nc.tensor.matmul(
                            s12[:, 0, :], lhsT=ones128_bf[:], rhs=src_bf[:, c8, :],
                            start=(c8 == 0), stop=(c8 == 7),
                        )
                        nc.tensor.matmul(
                            s12[:, 1, :], lhsT=ones128_bf[:], rhs=sq[:, c8, :],
                            start=(c8 == 0), stop=(c8 == 7),
                        )
                    stats = lnp.tile([1, 6, TOK], dt.float32, tag=f"st_{tag}")
                    mu = stats[:, 0, :]
                    m2 = stats[:, 1, :]
                    var = stats[:, 2, :]
                    rv = stats[:, 3, :]
                    rs = stats[:, 4, :]
                    musr = stats[:, 5, :]
                    nc.vector.tensor_scalar(mu, s12[:, 0, :], 1.0 / D, None, OP.mult)
                    nc.vector.tensor_scalar(m2, s12[:, 1, :], 1.0 / D, LN_EPS,
                                            OP.mult, OP.add)
                    nc.vector.tensor_tensor(var, mu, mu, OP.mult)
                    nc.vector.tensor_tensor(var, m2, var, OP.subtract)
                    nc.vector.reciprocal_approx_fast(rv, var)
                    nc.scalar.activation(rs, rv, AF.Sqrt)
                    nc.vector.tensor_tensor(musr, mu, rs, OP.mult)
                    rs_rep = lnp.tile([P, TOK], dt.float32, tag=f"rsr_{tag}")
                    nc.gpsimd.partition_broadcast(rs_rep[:], rs)
                    t = lnp.tile([P, TOK], dt.float32, tag=f"t_{tag}")
                    for c8 in range(8):
                        # shift = g_col(c8) (x) (mu * rs)   [P, TOK] in PSUM
                        sh = lnsh.tile([P, TOK], dt.float32, tag=f"sh_{tag}")
                        nc.tensor.matmul(
                            sh[:],
                            lhsT=gcol_sb[0:1, gcol_off + c8 * P:
                                         gcol_off + (c8 + 1) * P],
                            rhs=musr,
                            start=True, stop=True,
                        )
                        # t = (src * g_p) * rs_rep ; dst = (t + b_p) - shift
                        nc.vector.scalar_tensor_tensor(
                            t[:], src[:, c8, :], g_sb[:, c8:c8 + 1], rs_rep[:],
                            op0=OP.mult, op1=OP.mult,
                        )
                        nc.vector.scalar_tensor_tensor(
                            dst[:, c8, :], t[:], b_sb[:, c8:c8 + 1], sh[:],
                            op0=OP.add, op1=OP.subtract,
                        )

            pN_cm = tc.tile_pool(name="pN", bufs=1)
            pN = pN_cm.__enter__()
            n_bf = pN.tile([P, 8, TOK], dt.bfloat16, tag="n_bf")
            h1g = pN.tile([P, 32, TOK], dt.bfloat16, tag="h1g")
            r1bf = pN.tile([P, 8, TOK], dt.bfloat16, tag="r1bf")
            sq1 = pN.tile([P, 8, TOK], dt.bfloat16, tag="sq1")
            gcol_sb = pN.tile([1, 2 * D], dt.float32, tag="gcol")
            nc.gpsimd.dma_start(gcol_sb[:], gcol_d[:])
            for oc in range(8):
                nc.gpsimd.tensor_copy(r1bf[:, oc, :], r1[:, oc, :])
                nc.gpsimd.tensor_tensor(
                    sq1[:, oc, :], r1[:, oc, :], r1[:, oc, :], OP.mult
                )

            layernorm(r1, r1bf, sq1, n_bf, g1_sb, b1_sb, 0, "ln1")

            # prewarm Gelu table while LN1's normalize runs on DVE
            nc.scalar.activation(sq_warm[:], sq_warm[:], AF.Gelu)

            # ============ phase 5: MLP ============
            with (
                tc.tile_pool(name="wfc", bufs=2) as wfcp,
                tc.tile_pool(name="fc_ps", bufs=4, space="PSUM") as fc_ps,
            ):
                for oq in range(4):
                    wfq = wfcp.tile([P, 8, D], dt.bfloat16, tag="wfcq")
                    nc.sync.dma_start(wfq[:], wfc_d[:, :, oq * D:(oq + 1) * D])
                    for oc8 in range(8):
                        oc = oq * 8 + oc8
                        ps = fc_ps.tile([P, TOK], dt.float32, tag="fcps")
                        for c8 in range(8):
                            nc.tensor.matmul(
                                ps[:],
                                lhsT=wfq[:, c8, oc8 * P:(oc8 + 1) * P],
                                rhs=n_bf[:, c8, :],
                                start=(c8 == 0),
                                stop=(c8 == 7),
                            )
                        nc.scalar.activation(
                            h1g[:, oc, :], ps[:], AF.Gelu,
                            bias=bfc_sb[:, oc:oc + 1],
                        )

            r2 = r1  # reuse r1's tile for the second residual
            r2bf = r1bf
            sq2 = sq1
            with (
                tc.tile_pool(name="wpr", bufs=2) as wprp,
                tc.tile_pool(name="pr_ps", bufs=1, space="PSUM") as pr_ps,
            ):
                mps = [pr_ps.tile([P, TOK], dt.float32, tag=f"mps{i}",
                                  name=f"mps{i}") for i in range(8)]
                for q4 in range(4):
                    wpq = wprp.tile([P, 8, D], dt.bfloat16, tag="wprq")
                    nc.sync.dma_start(wpq[:], wpr_d[:, 8 * q4:8 * (q4 + 1), :])
                    for oc in range(8):
                        for c8 in range(8):
                            nc.tensor.matmul(
                                mps[oc][:],
                                lhsT=wpq[:, c8, oc * P:(oc + 1) * P],
                                rhs=h1g[:, q4 * 8 + c8, :],
                                start=(q4 == 0 and c8 == 0),
                                stop=(q4 == 3 and c8 == 7),
                            )
                # prewarm Sqrt for LN2 (ACT idle now)
                nc.scalar.activation(sq_warm[:], sq_warm[:], AF.Sqrt)
                for oc in range(8):
                    # r2 = mps + bpr + n   (n in bf16)
                    nc.vector.scalar_tensor_tensor(
                        r2[:, oc, :], mps[oc][:], bpr_sb[:, oc:oc + 1],
                        n_bf[:, oc, :], op0=OP.add, op1=OP.add,
                    )
                    nc.gpsimd.tensor_copy(r2bf[:, oc, :], r2[:, oc, :])
                    nc.gpsimd.tensor_tensor(
                        sq2[:, oc, :], r2[:, oc, :], r2[:, oc, :], OP.mult
                    )

            layernorm(r2, r2bf, sq2, r2, g2_sb, b2_sb, D, "ln2")
            out_v = out_d.rearrange("(c p) t -> p c t", p=P)
            for c8 in range(8):
                nc.sync.dma_start(out_v[:, c8, :], r2[:, c8, :])
            pN_cm.__exit__(None, None, None)
            pR_cm.__exit__(None, None, None)

    nc.compile()
    return nc


def _prep_shared(w_attn, b_attn, w_o, b_o, ln1_g, ln1_b, w_fc, b_fc, w_pr, b_pr,
                 ln2_g, ln2_b):
    w_attn = np.asarray(w_attn, np.float32)
    b_attn = np.asarray(b_attn, np.float32)
    w_o_f = np.asarray(w_o, np.float32)
    b_v = b_attn[2 * D:]
    b_o_eff = (np.asarray(b_o, np.float32) + b_v @ w_o_f).astype(np.float32)
    mask = np.where(
        np.arange(P)[:, None] > np.arange(P)[None, :], MASK_NEG, 0.0
    ).astype(BF16)  # [ki, qj]: mask keys above the diagonal

    def sb(w):  # [D, Fo] -> [P, 8, Fo]  (feature-chunk-major SBUF layout)
        Fo = w.shape[1]
        return np.ascontiguousarray(
            w.reshape(8, P, Fo).transpose(1, 0, 2)).astype(BF16)

    bias_pack = np.zeros((P, 96), np.float32)
    bias_pack[:, 0:16] = b_attn[:2 * D].reshape(16, P).T
    bias_pack[:, 16:24] = b_o_eff.reshape(8, P).T
    bias_pack[:, 24:56] = np.asarray(b_fc, np.float32).reshape(32, P).T
    bias_pack[:, 56:64] = np.asarray(b_pr, np.float32).reshape(8, P).T
    bias_pack[:, 64:72] = np.asarray(ln1_g, np.float32).reshape(8, P).T
    bias_pack[:, 72:80] = np.asarray(ln1_b, np.float32).reshape(8, P).T
    bias_pack[:, 80:88] = np.asarray(ln2_g, np.float32).reshape(8, P).T
    bias_pack[:, 88:96] = np.asarray(ln2_b, np.float32).reshape(8, P).T

    gcol = np.concatenate([np.asarray(ln1_g, np.float32),
                           np.asarray(ln2_g, np.float32)])[None, :]

    im = np.stack([np.eye(P, dtype=np.float32).astype(BF16), mask], axis=1)

    wpr_f = np.asarray(w_pr, np.float32)  # [F, D]
    shared = {
        "wq_d": sb(w_attn[:, 0:D]),
        "wk_d": sb(w_attn[:, D:2 * D]),
        "wv_d": sb(np.ascontiguousarray(w_attn[:, 2 * D:])),
        "wo_d": sb(w_o_f),
        "wfc_d": sb(np.asarray(w_fc, np.float32)),
        "wpr_d": np.ascontiguousarray(
            wpr_f.reshape(32, P, D).transpose(1, 0, 2)).astype(BF16),
        "bias_d": bias_pack,
        "gcol_d": np.ascontiguousarray(gcol),
        "im_d": np.ascontiguousarray(im),
        "sel8": np.repeat(np.eye(8, dtype=np.float32), 64, axis=1).astype(BF16),
    }
    return shared


def kernel(x, w_attn, b_attn, w_o, b_o, ln1_g, ln1_b, w_fc, b_fc, w_pr, b_pr,
           ln2_g, ln2_b, _trace=False):
    from concourse.bass_utils import run_bass_kernel_spmd

    if "nc" not in _CACHE:
        _CACHE["nc"] = _build()
    nc = _CACHE["nc"]

    x = np.asarray(x, np.float32)
    shared = _prep_shared(w_attn, b_attn, w_o, b_o, ln1_g, ln1_b, w_fc, b_fc,
                          w_pr, b_pr, ln2_g, ln2_b)

    in_maps = []
    idxs = []
    for c in range(N_CORES):
        b, j = c // 4, c % 4
        idx = np.r_[j * 256:(j + 1) * 256, (7 - j) * 256:(8 - j) * 256]
        idxs.append((b, idx))
        xT = np.ascontiguousarray(x[b, idx, :].T)  # [D, TOK]
        xs = xT.reshape(8, P, TOK).transpose(1, 0, 2)  # [P, 8, TOK]
        m = dict(shared)
        m["x_bf"] = np.ascontiguousarray(xs.astype(BF16))
        m["jidx"] = np.array([[j]], np.uint32)
        in_maps.append(m)

    res = run_bass_kernel_spmd(
        nc, in_maps, core_ids=list(range(N_CORES)), trace=_trace
    )
    if _trace:
        _CACHE["exec_time_ns"] = res.exec_time_ns
        it = getattr(res, "instructions_and_trace", None)
        _CACHE["trace_path"] = it[1] if it else None

    out = np.empty((B, S, D), np.float32)
    for c in range(N_CORES):
        b, idx = idxs[c]
        out[b, idx, :] = res.results[c]["out"].T
    return out


# revision 33
# speedup vs baseline: 1.1305x; 1.1305x over previous
"""Dense transformer block (attention + post-LN MLP) on 8 trn2 NeuronCores.

Context-parallel sharding: core c handles batch c//4 and token blocks
j*256, (7-j)*256 of that batch (j = c%4) -> causally balanced 512
tokens/core. Weights replicated (bf16).

v2 structure:
 - Host ships every tensor in exact SBUF layout (dense per-partition
   lines, one DMA each); the big phase-1 loads are issued first so the
   PE starts early.
 - K is projected FIRST and shipped as its own AllGather; V follows as
   two half-AllGathers (heads 0-7, 8-15); Q is projected while the
   rings run. Remote attention unblocks progressively.
 - Attention is two-pass: ALL head-pairs' local chunks run during the
   rings (partial osum evacuation), then the remote pass combines into
   osum. Deferred softmax normalization in two halves (hp 0-3 while
   hp 4-7 still compute) using reciprocal_approx_fast + GPSIMD
   partition_broadcast.
 - LayerNorms: bf16 cast + square fused into the producer phase,
   column-sum matmuls back-to-back, g-folded 2-op normalize, prewarmed
   Sqrt/Gelu tables. Residuals use bf16 x/n (tolerance 2e-2).
"""

import numpy as np
import ml_dtypes

BF16 = ml_dtypes.bfloat16

N_CORES = 8
B, S, D = 2, 2048, 1024
H, HD = 16, 64
F = 4 * D
TOK = 512            # tokens per core
P = 128
NHP = H // 2         # 8 head pairs
MASK_NEG = -80000.0  # -> -79872 in bf16; /8 => exp underflows to exactly 0
LN_EPS = 1e-5

VW = 66              # per-head V row width: 64 v + ones col + pad
KT_ELEMS = P * 8 * TOK           # K payload per core
V_ELEMS = P * 4 * (16 * VW)      # V payload per core

_CACHE = {}


def _q_blocks(j):
    """Global 128-blocks of this core's query tokens, ascending."""
    return [2 * j, 2 * j + 1, 14 - 2 * j, 15 - 2 * j]


def _remote_plan(j):
    """Remote chunks in assumed ring-arrival order (ranks j-1, j-2, j-3),
    each rank's valid chunks ascending.  Returns (mq, [(kc, slot, lt)],
    rank_of_slot)."""
    gq = _q_blocks(j)
    last_kc = gq[-1]
    plan = []
    rank_of_slot = []
    for d in range(1, 4):
        r = (j - d) % 4
        rb = _q_blocks(r)
        got = False
        for lt, kc in enumerate(rb):
            if kc <= last_kc and kc not in gq:
                plan.append((kc, len(rank_of_slot), lt))
                got = True
        if got:
            rank_of_slot.append(r)
    mq = 256 if j == 0 else 0
    return mq, plan, rank_of_slot


def _build():
    import concourse.bass as bass
    import concourse.mybir as mybir
    import concourse.tile as tile
    from concourse import bacc

    dt = mybir.dt
    AF = mybir.ActivationFunctionType
    OP = mybir.AluOpType

    nc = bacc.Bacc(
        "TRN2",
        target_bir_lowering=False,
        debug=False,
        enable_asserts=True,
        num_devices=N_CORES,
    )

    def din(name, shape, dty):
        return nc.dram_tensor(name, shape, dty, kind="ExternalInput").ap()

    # host ships everything in SBUF-exact layout
    x_bf = din("x_bf", [P, 8, TOK], dt.bfloat16)
    wq_d = din("wq_d", [P, 8, D], dt.bfloat16)     # Q columns of w_attn
    wk_d = din("wk_d", [P, 8, D], dt.bfloat16)     # K columns
    wv_d = din("wv_d", [P, 8, D], dt.bfloat16)     # V columns
    wo_d = din("wo_d", [P, 8, D], dt.bfloat16)
    wfc_d = din("wfc_d", [P, 8, F], dt.bfloat16)
    wpr_d = din("wpr_d", [P, 32, D], dt.bfloat16)
    # bias pack cols: bqk 16 | bo 8 | bfc 32 | bpr 8 | g1 8 | b1 8 | g2 8 | b2 8
    bias_d = din("bias_d", [P, 96], dt.float32)
    gcol_d = din("gcol_d", [1, 2 * D], dt.float32)  # g1 | g2 by feature
    im_d = din("im_d", [P, 2, P], dt.bfloat16)      # ident | maskm
    sel8_d = din("sel8", [8, 8 * 64], dt.bfloat16)  # one-hot row selectors
    jidx_d = din("jidx", [1, 1], dt.uint32)
    out_d = nc.dram_tensor("out", [D, TOK], dt.float32, kind="ExternalOutput").ap()

    with tile.TileContext(nc) as tc:
        from contextlib import ExitStack

        ctx = ExitStack()
        with ctx:
            c_pool = ctx.enter_context(tc.tile_pool(name="consts", bufs=1))
            dram = ctx.enter_context(tc.tile_pool(name="dram", bufs=1, space="DRAM"))

            # ---- long-lived pools (stack: pR > kvp > pA > gat > transient) ----
            pR_cm = tc.tile_pool(name="pR", bufs=1)
            pR = pR_cm.__enter__()
            kvp_cm = tc.tile_pool(name="kv_own", bufs=1)
            kvp = kvp_cm.__enter__()
            pA_cm = tc.tile_pool(name="pA", bufs=1)
            pA = pA_cm.__enter__()
            gat_cm = tc.tile_pool(name="gat", bufs=1)
            gat = gat_cm.__enter__()

            # ---- phase-1 critical loads FIRST (sync engine), split in halves
            # so the first matmuls can start before the full tiles land ----
            xb = pA.tile([P, 8, TOK], dt.bfloat16, tag="xb")
            xwk_cm = tc.tile_pool(name="xwk", bufs=2)
            xwk = xwk_cm.__enter__()
            wk = xwk.tile([P, 8, D], dt.bfloat16, tag="w8", name="wk")
            nc.sync.dma_start(xb[:, 0:4, :], x_bf[:, 0:4, :])
            nc.sync.dma_start(wk[:, 0:4, :], wk_d[:, 0:4, :])
            nc.sync.dma_start(xb[:, 4:8, :], x_bf[:, 4:8, :])
            nc.sync.dma_start(wk[:, 4:8, :], wk_d[:, 4:8, :])
            wv = xwk.tile([P, 8, D], dt.bfloat16, tag="w8", name="wv")
            nc.sync.dma_start(wv[:], wv_d[:])

            # ---- per-core j register (for Switch) ----
            jreg = nc.alloc_registers(
                "jreg",
                [mybir.EngineType.PE, mybir.EngineType.Activation,
                 mybir.EngineType.DVE, mybir.EngineType.SP,
                 mybir.EngineType.Pool],
            )
            nc.regs_load(jreg, jidx_d[0:1, 0:1])
            jval = nc.snap(jreg, donate=True, min_val=0, max_val=3)

            # ---- small consts (gpsimd queue keeps sync free) ----
            bias_sb = c_pool.tile([P, 96], dt.float32, tag="bias")
            nc.gpsimd.dma_start(bias_sb[:], bias_d[:])
            im_sb = c_pool.tile([P, 2, P], dt.bfloat16, tag="im")
            nc.gpsimd.dma_start(im_sb[:], im_d[:])
            ones128_bf = c_pool.tile([P, 1], dt.bfloat16, tag="ones128")
            nc.vector.memset(ones128_bf[:], 1.0)
            sel8 = c_pool.tile([8, 8, 64], dt.bfloat16, tag="sel8")
            nc.gpsimd.dma_start(
                sel8[:], sel8_d.rearrange("p (c f) -> p c f", f=64))
            sq_warm = c_pool.tile([1, 1], dt.float32, tag="sqwarm")
            nc.vector.memset(sq_warm[:], 1.0)

            bqk_sb = bias_sb[:, 0:16]
            bo_sb = bias_sb[:, 16:24]
            bfc_sb = bias_sb[:, 24:56]
            bpr_sb = bias_sb[:, 56:64]
            g1_sb = bias_sb[:, 64:72]
            b1_sb = bias_sb[:, 72:80]
            g2_sb = bias_sb[:, 80:88]
            b2_sb = bias_sb[:, 88:96]
            ident = im_sb[:, 0, :]
            maskm = im_sb[:, 1, :]

            # ---- AllGather buffers (fp8 payload, moved as uint8 bytes) ----
            agk_in = dram.tile([KT_ELEMS], dt.uint8, tag="agki", name="agki")
            agk_out = dram.tile([4, KT_ELEMS], dt.uint8, tag="agko", name="agko")
            agv_in = dram.tile([V_ELEMS], dt.uint8, tag="agvi", name="agvi")
            agv_out = dram.tile([4, V_ELEMS], dt.uint8, tag="agvo", name="agvo")
            GROUPS = [[0, 1, 2, 3], [4, 5, 6, 7]]

            # ---- long-lived SBUF tiles ----
            r1 = pR.tile([P, 8, TOK], dt.float32, tag="r1")

            v_own = kvp.tile([P, 4, 16 * VW], dt.bfloat16, tag="v_own")
            kT_own = kvp.tile([P, 8, TOK], dt.bfloat16, tag="kT_own")
            qT = kvp.tile([P, 8, TOK], dt.bfloat16, tag="qT")
            # fp8 copies: K/V for the collectives, Q for remote-chunk matmuls
            kT_f8 = kvp.tile([P, 8, TOK], dt.float8e4, tag="kT_f8")
            v_f8 = kvp.tile([P, 4, 16 * VW], dt.float8e4, tag="v_f8")
            qT_f8 = kvp.tile([P, 8, TOK], dt.float8e4, tag="qT_f8")

            o_cat = [pA.tile([P, TOK], dt.bfloat16, tag=f"o_cat{i}", name=f"o_cat{i}")
                     for i in range(NHP)]
            wo_sb = pA.tile([P, 8, D], dt.bfloat16, tag="wo")

            kT_s = [gat.tile([P, 8, TOK], dt.float8e4, tag=f"kTs{s}", name=f"kTs{s}")
                    for s in range(3)]
            v_s = [gat.tile([P, 4, 16 * VW], dt.float8e4, tag=f"vs{s}",
                            name=f"vs{s}") for s in range(3)]
            osum = [gat.tile([65, 2, TOK], dt.bfloat16, tag=f"osum{i}",
                             name=f"osum{i}") for i in range(NHP)]
            dcath = [gat.tile([4, 2, TOK], dt.bfloat16, tag=f"dcat{i}",
                              name=f"dcat{i}") for i in range(2)]
            dcat_f = gat.tile([4, 2, TOK], dt.float32, tag="dcat_f")
            rcat_f = gat.tile([4, 2, TOK], dt.float32, tag="rcat_f")
            rcath = [gat.tile([4, 2, TOK], dt.bfloat16, tag=f"rcat{i}",
                              name=f"rcat{i}") for i in range(2)]

            # ============ phase 1: K -> ship, V -> ship x2, Q ============
            with tc.tile_pool(name="qkv_ps", bufs=4, space="PSUM") as qkv_ps:

                def qk_quarter(w_sb, qi, dest, kbias, f8dest):
                    for fo in range(4):
                        fchunk = 4 * qi + fo
                        ps = qkv_ps.tile([P, TOK], dt.float32, tag="qkvps")
                        for c8 in range(8):
                            nc.tensor.matmul(
                                ps[:],
                                lhsT=w_sb[:, c8, qi * TOK + fo * P:
                                          qi * TOK + (fo + 1) * P],
                                rhs=xb[:, c8, :],
                                start=(c8 == 0),
                                stop=(c8 == 7),
                            )
                        bcol = fchunk + (8 if kbias else 0)
                        nc.vector.tensor_scalar(
                            dest[:, fchunk, :], ps[:],
                            bqk_sb[:, bcol:bcol + 1], None, OP.add,
                        )
                        nc.vector.tensor_copy(
                            f8dest[:, fchunk, :], dest[:, fchunk, :]
                        )

                # K first; ship its fp8 copy from the vector queue (the f8
                # copies precede it there, so deps resolve in FIFO order)
                qk_quarter(wk, 0, kT_own, True, kT_f8)
                qk_quarter(wk, 1, kT_own, True, kT_f8)
                nc.scalar.dma_start(
                    agk_in.rearrange("(c p t) -> p c t", c=8, p=P)
                    .bitcast(dt.float8e4),
                    kT_f8[:],
                )
                nc.gpsimd.collective_compute(
                    "AllGather", mybir.AluOpType.bypass,
                    replica_groups=GROUPS,
                    ins=[agk_in.opt()], outs=[agk_out.opt()],
                )
                # wq rotates into wk's slot once the K quarters are done
                wq = xwk.tile([P, 8, D], dt.bfloat16, tag="w8", name="wq")
                nc.sync.dma_start(wq[:], wq_d[:])

                # V next
                v4 = v_own.rearrange("p c (h w) -> p c h w", w=VW)
                v4_f8 = v_f8.rearrange("p c (h w) -> p c h w", w=VW)
                nc.vector.memset(v4[:, :, :, 64:66], 0.0)
                nc.vector.memset(v4[:, :, :, 64:65], 1.0)
                nc.vector.memset(v4_f8[:, :, :, 64:66], 0.0)
                nc.vector.memset(v4_f8[:, :, :, 64:65], 1.0)
                for t4 in range(4):
                    for vc in range(2):
                        ps = qkv_ps.tile([P, TOK], dt.float32, tag="qkvps")
                        for c8 in range(8):
                            nc.tensor.matmul(
                                ps[:],
                                lhsT=xb[:, c8, t4 * P:(t4 + 1) * P],
                                rhs=wv[:, c8, vc * TOK:(vc + 1) * TOK],
                                start=(c8 == 0),
                                stop=(c8 == 7),
                            )
                        dest = v4[:, t4, vc * 8:(vc + 1) * 8, 0:64]
                        nc.vector.tensor_copy(
                            dest, ps[:].rearrange("p (h w) -> p h w", w=64)
                        )
                        nc.vector.tensor_copy(
                            v4_f8[:, t4, vc * 8:(vc + 1) * 8, 0:64], dest
                        )
                nc.scalar.dma_start(
                    agv_in.rearrange("(c p f) -> p c f", c=4, p=P)
                    .bitcast(dt.float8e4),
                    v_f8[:],
                )
                nc.gpsimd.collective_compute(
                    "AllGather", mybir.AluOpType.bypass,
                    replica_groups=GROUPS,
                    ins=[agv_in.opt()], outs=[agv_out.opt()],
                )

                # Q last (overlaps the rings)
                qk_quarter(wq, 0, qT, False, qT_f8)
                qk_quarter(wq, 1, qT, False, qT_f8)

            xwk_cm.__exit__(None, None, None)

            # prefetch w_o for the post-attention projection
            nc.sync.dma_start(wo_sb[:], wo_d[:])

            # ============ attention ============
            with (
                tc.tile_pool(name="att_s", bufs=2, space="PSUM") as s_ps_pool,
                tc.tile_pool(name="att_o", bufs=2, space="PSUM") as o_ps_pool,
                tc.tile_pool(name="att_tmp", bufs=3) as atmp,
                tc.tile_pool(name="att_nrm", bufs=2) as anrm,
            ):
                def attn_qkav(hp, kc, gq, oT, first, last, ksrc, vsrc, f8):
                    vi = [i for i in range(4) if gq[i] >= kc]
                    qs = P * vi[0]
                    diag = gq[vi[0]] == kc
                    qsrc = qT_f8 if f8 else qT
                    sps = s_ps_pool.tile([P, 2, TOK], dt.float32, tag="sps")
                    for h2 in range(2):
                        lo, hi = 64 * h2, 64 * (h2 + 1)
                        nc.tensor.matmul(
                            sps[:, h2, qs:],
                            lhsT=ksrc[lo:hi, :],
                            rhs=qsrc[lo:hi, hp, qs:],
                            start=True,
                            stop=not diag,
                            tile_position=(64 * h2, 0),
                        )
                        if diag:
                            nc.tensor.matmul(
                                sps[:, h2, qs:qs + P],
                                lhsT=ident[:],
                                rhs=maskm[:],
                                start=False,
                                stop=True,
                            )
                    if f8:
                        aT = atmp.tile([P, 2, TOK], dt.float8e4, tag="aT8")
                    else:
                        aT = atmp.tile([P, 2, TOK], dt.bfloat16, tag="aT")
                    nc.scalar.activation(
                        aT[:, :, qs:], sps[:, :, qs:], AF.Exp, scale=0.125
                    )
                    for h2 in range(2):
                        h = 2 * hp + h2
                        nc.tensor.matmul(
                            oT[:, h2, qs:],
                            lhsT=vsrc[:, h * VW:h * VW + 65],
                            rhs=aT[:, h2, qs:],
                            start=first,
                            stop=last,
                        )

                def norm_half(h0):
                    """Deferred softmax normalization for hps h0..h0+3."""
                    half = h0 // 4
                    rc = rcath[half]
                    nc.vector.tensor_copy(dcat_f[:], dcath[half][:])
                    nc.vector.reciprocal_approx_fast(rcat_f[:], dcat_f[:])
                    nc.vector.tensor_copy(rc[:], rcat_f[:])
                    for hp in range(h0, h0 + 4):
                        rep = s_ps_pool.tile([P, 2, TOK], dt.float32, tag="sps")
                        for h2 in range(2):
                            nc.tensor.matmul(
                                rep[0:64, h2, :],
                                lhsT=sel8[0:4, hp - h0, :],
                                rhs=rc[:, h2, :],
                                start=True, stop=True,
                            )
                        ot = anrm.tile([64, TOK], dt.bfloat16, tag="otmp")
                        nc.vector.tensor_tensor(
                            o_cat[hp][0:64, :], osum[hp][0:64, 0, :],
                            rep[0:64, 0, :], OP.mult,
                        )
                        nc.vector.tensor_tensor(
                            ot[:], osum[hp][0:64, 1, :], rep[0:64, 1, :], OP.mult,
                        )
                        nc.sync.dma_start(o_cat[hp][64:128, :], ot[:])

                def attn_arm(j):
                    gq = _q_blocks(j)
                    mq, plan, rank_of_slot = _remote_plan(j)
                    # monotone dependency order on sync: K loads (ready
                    # first) then V loads -- no head-of-line blocking
                    for s, r in enumerate(rank_of_slot):
                        nc.sync.dma_start(
                            kT_s[s][:],
                            agk_out[r].rearrange("(c p t) -> p c t", c=8, p=P)
                            .bitcast(dt.float8e4),
                        )
                    for s, r in enumerate(rank_of_slot):
                        nc.sync.dma_start(
                            v_s[s][:],
                            agv_out[r].rearrange("(c p f) -> p c f", c=4, p=P)
                            .bitcast(dt.float8e4),
                        )
                    # pass 1: local chunks for ALL hps (overlaps the rings)
                    for hp in range(NHP):
                        oT = o_ps_pool.tile([65, 2, TOK], dt.float32, tag="oT")
                        for ki, kc in enumerate(gq):
                            li = gq.index(kc)
                            attn_qkav(hp, kc, gq, oT, ki == 0, ki == 3,
                                      kT_own[:, hp, li * P:(li + 1) * P],
                                      v_own[:, li, :], False)
                        nc.vector.tensor_copy(osum[hp][:], oT[:])
                    # pass 2: remote chunks, combine
                    for hp in range(NHP):
                        oT = o_ps_pool.tile([65, 2, TOK], dt.float32, tag="oT")
                        for ki, (kc, s, lt) in enumerate(plan):
                            attn_qkav(hp, kc, gq, oT, ki == 0,
                                      ki == len(plan) - 1,
                                      kT_s[s][:, hp, lt * P:(lt + 1) * P],
                                      v_s[s][:, lt, :], True)
                        nc.vector.tensor_tensor(
                            osum[hp][:, :, mq:], osum[hp][:, :, mq:],
                            oT[:, :, mq:], OP.add,
                        )
                        nc.sync.dma_start(
                            dcath[hp // 4][hp % 4:hp % 4 + 1, :, :],
                            osum[hp][64:65, :, :])
                        if hp == 3:
                            norm_half(0)
                    norm_half(4)

                for case in tc.Switch(jval, 4):
                    attn_arm(case)

            gat_cm.__exit__(None, None, None)

            # prewarm Sqrt table for LN1 (ACT idle during w_o)
            nc.scalar.activation(sq_warm[:], sq_warm[:], AF.Sqrt)

            # ============ phase 4: w_o + residual -> r1 (+ LN1 prep) ============
            with tc.tile_pool(name="wo_ps", bufs=4, space="PSUM") as wo_ps:
                for oc in range(8):
                    ps = wo_ps.tile([P, TOK], dt.float32, tag="wops")
                    for hp in range(8):
                        nc.tensor.matmul(
                            ps[:],
                            lhsT=wo_sb[:, hp, oc * P:(oc + 1) * P],
                            rhs=o_cat[hp][:],
                            start=(hp == 0),
                            stop=(hp == 7),
                        )
                    nc.vector.scalar_tensor_tensor(
                        r1[:, oc, :], ps[:], bo_sb[:, oc:oc + 1], xb[:, oc, :],
                        op0=OP.add, op1=OP.add,
                    )

            pA_cm.__exit__(None, None, None)
            kvp_cm.__exit__(None, None, None)

            # ============ layernorm (feature-major, partition reduce) ============
            def layernorm(src, src_bf, sq, dst, g_sb, b_sb, gcol_off, tag):
                """dst[:, c8, :] = LN(src)[c8] * g + b.  src fp32 [P,8,TOK];
                src_bf/sq bf16 precomputed.  dst may alias src (per-c8 safe)."""
                with (
                    tc.tile_pool(name=f"ln_{tag}", bufs=1) as lnp,
                    tc.tile_pool(name=f"lnps_{tag}", bufs=1, space="PSUM") as lnps,
                    tc.tile_pool(name=f"lnsh_{tag}", bufs=2, space="PSUM") as lnsh,
                ):
                    s12 = lnps.tile([1, 2, TOK], dt.float32, tag=f"s12_{tag}")
                    for c8 in range(8):
                        nc.tensor.matmul(
                            s12[:, 0, :], lhsT=ones128_bf[:], rhs=src_bf[:, c8, :],
                            start=(c8 == 0), stop=(c8 == 7),
                        )
                        nc.tensor.matmul(
                            s12[:, 1, :], lhsT=ones128_bf[:], rhs=sq[:, c8, :],
                            start=(c8 == 0), stop=(c8 == 7),
                        )
                    stats = lnp.tile([1, 6, TOK], dt.float32, tag=f"st_{tag}")
                    mu = stats[:, 0, :]
                    m2 = stats[:, 1, :]
                    var = stats[:, 2, :]
                    rv = stats[:, 3, :]
                    rs = stats[:, 4, :]
                    musr = stats[:, 5, :]
                    nc.vector.tensor_scalar(mu, s12[:, 0, :], 1.0 / D, None, OP.mult)
                    nc.vector.tensor_scalar(m2, s12[:, 1, :], 1.0 / D, LN_EPS,
                                            OP.mult, OP.add)
                    nc.vector.tensor_tensor(var, mu, mu, OP.mult)
                    nc.vector.tensor_tensor(var, m2, var, OP.subtract)
                    nc.vector.reciprocal_approx_fast(rv, var)
                    nc.scalar.activation(rs, rv, AF.Sqrt)
                    nc.vector.tensor_tensor(musr, mu, rs, OP.mult)
                    rs_rep = lnp.tile([P, TOK], dt.float32, tag=f"rsr_{tag}")
                    nc.gpsimd.partition_broadcast(rs_rep[:], rs)
                    t = lnp.tile([P, TOK], dt.float32, tag=f"t_{tag}")
                    for c8 in range(8):
                        # shift = g_col(c8) (x) (mu * rs)   [P, TOK] in PSUM
                        sh = lnsh.tile([P, TOK], dt.float32, tag=f"sh_{tag}")
                        nc.tensor.matmul(
                            sh[:],
                            lhsT=gcol_sb[0:1, gcol_off + c8 * P:
                                         gcol_off + (c8 + 1) * P],
                            rhs=musr,
                            start=True, stop=True,
                        )
                        # t = (src * g_p) * rs_rep ; dst = (t + b_p) - shift
                        nc.vector.scalar_tensor_tensor(
                            t[:], src[:, c8, :], g_sb[:, c8:c8 + 1], rs_rep[:],
                            op0=OP.mult, op1=OP.mult,
                        )
                        nc.vector.scalar_tensor_tensor(
                            dst[:, c8, :], t[:], b_sb[:, c8:c8 + 1], sh[:],
                            op0=OP.add, op1=OP.subtract,
                        )

            pN_cm = tc.tile_pool(name="pN", bufs=1)
            pN = pN_cm.__enter__()
            n_bf = pN.tile([P, 8, TOK], dt.bfloat16, tag="n_bf")
            h1g = pN.tile([P, 32, TOK], dt.bfloat16, tag="h1g")
            r1bf = pN.tile([P, 8, TOK], dt.bfloat16, tag="r1bf")
            sq1 = pN.tile([P, 8, TOK], dt.bfloat16, tag="sq1")
            gcol_sb = pN.tile([1, 2 * D], dt.float32, tag="gcol")
            nc.gpsimd.dma_start(gcol_sb[:], gcol_d[:])
            for oc in range(8):
                nc.gpsimd.tensor_copy(r1bf[:, oc, :], r1[:, oc, :])
                nc.gpsimd.tensor_tensor(
                    sq1[:, oc, :], r1[:, oc, :], r1[:, oc, :], OP.mult
                )

            layernorm(r1, r1bf, sq1, n_bf, g1_sb, b1_sb, 0, "ln1")

            # prewarm Gelu table while LN1's normalize runs on DVE
            nc.scalar.activation(sq_warm[:], sq_warm[:], AF.Gelu)

            # ============ phase 5: MLP ============
            with (
                tc.tile_pool(name="wfc", bufs=2) as wfcp,
                tc.tile_pool(name="fc_ps", bufs=4, space="PSUM") as fc_ps,
            ):
                for oq in range(4):
                    wfq = wfcp.tile([P, 8, D], dt.bfloat16, tag="wfcq")
                    nc.sync.dma_start(wfq[:], wfc_d[:, :, oq * D:(oq + 1) * D])
                    for oc8 in range(8):
                        oc = oq * 8 + oc8
                        ps = fc_ps.tile([P, TOK], dt.float32, tag="fcps")
                        for c8 in range(8):
                            nc.tensor.matmul(
                                ps[:],
                                lhsT=wfq[:, c8, oc8 * P:(oc8 + 1) * P],
                                rhs=n_bf[:, c8, :],
                                start=(c8 == 0),
                                stop=(c8 == 7),
                            )
                        nc.scalar.activation(
                            h1g[:, oc, :], ps[:], AF.Gelu,
                            bias=bfc_sb[:, oc:oc + 1],
                        )

            r2 = r1  # reuse r1's tile for the second residual
            r2bf = r1bf
            sq2 = sq1
            with (
                tc.tile_pool(name="wpr", bufs=2) as wprp,
                tc.tile_pool(name="pr_ps", bufs=1, space="PSUM") as pr_ps,
            ):
                mps = [pr_ps.tile([P, TOK], dt.float32, tag=f"mps{i}",
                                  name=f"mps{i}") for i in range(8)]
                for q4 in range(4):
                    wpq = wprp.tile([P, 8, D], dt.bfloat16, tag="wprq")
                    nc.sync.dma_start(wpq[:], wpr_d[:, 8 * q4:8 * (q4 + 1), :])
                    for oc in range(8):
                        for c8 in range(8):
                            nc.tensor.matmul(
                                mps[oc][:],
                                lhsT=wpq[:, c8, oc * P:(oc + 1) * P],
                                rhs=h1g[:, q4 * 8 + c8, :],
                                start=(q4 == 0 and c8 == 0),
                                stop=(q4 == 3 and c8 == 7),
                            )
                # prewarm Sqrt for LN2 (ACT idle now)
                nc.scalar.activation(sq_warm[:], sq_warm[:], AF.Sqrt)
                for oc in range(8):
                    # r2 = mps + bpr + n   (n in bf16)
                    nc.vector.scalar_tensor_tensor(
                        r2[:, oc, :], mps[oc][:], bpr_sb[:, oc:oc + 1],
                        n_bf[:, oc, :], op0=OP.add, op1=OP.add,
                    )
                    nc.gpsimd.tensor_copy(r2bf[:, oc, :], r2[:, oc, :])
                    nc.gpsimd.tensor_tensor(
                        sq2[:, oc, :], r2[:, oc, :], r2[:, oc, :], OP.mult
                    )

            layernorm(r2, r2bf, sq2, r2, g2_sb, b2_sb, D, "ln2")
            out_v = out_d.rearrange("(c p) t -> p c t", p=P)
            for c8 in range(8):
                nc.sync.dma_start(out_v[:, c8, :], r2[:, c8, :])
            pN_cm.__exit__(None, None, None)
            pR_cm.__exit__(None, None, None)

    nc.compile()
    return nc


def _prep_shared(w_attn, b_attn, w_o, b_o, ln1_g, ln1_b, w_fc, b_fc, w_pr, b_pr,
                 ln2_g, ln2_b):
    w_attn = np.asarray(w_attn, np.float32)
    b_attn = np.asarray(b_attn, np.float32)
    w_o_f = np.asarray(w_o, np.float32)
    b_v = b_attn[2 * D:]
    b_o_eff = (np.asarray(b_o, np.float32) + b_v @ w_o_f).astype(np.float32)
    mask = np.where(
        np.arange(P)[:, None] > np.arange(P)[None, :], MASK_NEG, 0.0
    ).astype(BF16)  # [ki, qj]: mask keys above the diagonal

    def sb(w):  # [D, Fo] -> [P, 8, Fo]  (feature-chunk-major SBUF layout)
        Fo = w.shape[1]
        return np.ascontiguousarray(
            w.reshape(8, P, Fo).transpose(1, 0, 2)).astype(BF16)

    bias_pack = np.zeros((P, 96), np.float32)
    bias_pack[:, 0:16] = b_attn[:2 * D].reshape(16, P).T
    bias_pack[:, 16:24] = b_o_eff.reshape(8, P).T
    bias_pack[:, 24:56] = np.asarray(b_fc, np.float32).reshape(32, P).T
    bias_pack[:, 56:64] = np.asarray(b_pr, np.float32).reshape(8, P).T
    bias_pack[:, 64:72] = np.asarray(ln1_g, np.float32).reshape(8, P).T
    bias_pack[:, 72:80] = np.asarray(ln1_b, np.float32).reshape(8, P).T
    bias_pack[:, 80:88] = np.asarray(ln2_g, np.float32).reshape(8, P).T
    bias_pack[:, 88:96] = np.asarray(ln2_b, np.float32).reshape(8, P).T

    gcol = np.concatenate([np.asarray(ln1_g, np.float32),
                           np.asarray(ln2_g, np.float32)])[None, :]

    im = np.stack([np.eye(P, dtype=np.float32).astype(BF16), mask], axis=1)

    wpr_f = np.asarray(w_pr, np.float32)  # [F, D]
    shared = {
        "wq_d": sb(w_attn[:, 0:D]),
        "wk_d": sb(w_attn[:, D:2 * D]),
        "wv_d": sb(np.ascontiguousarray(w_attn[:, 2 * D:])),
        "wo_d": sb(w_o_f),
        "wfc_d": sb(np.asarray(w_fc, np.float32)),
        "wpr_d": np.ascontiguousarray(
            wpr_f.reshape(32, P, D).transpose(1, 0, 2)).astype(BF16),
        "bias_d": bias_pack,
        "gcol_d": np.ascontiguousarray(gcol),
        "im_d": np.ascontiguousarray(im),
        "sel8": np.repeat(np.eye(8, dtype=np.float32), 64, axis=1).astype(BF16),
    }
    return shared


def kernel(x, w_attn, b_attn, w_o, b_o, ln1_g, ln1_b, w_fc, b_fc, w_pr, b_pr,
           ln2_g, ln2_b, _trace=False):
    from concourse.bass_utils import run_bass_kernel_spmd

    if "nc" not in _CACHE:
        _CACHE["nc"] = _build()
    nc = _CACHE["nc"]

    x = np.asarray(x, np.float32)
    shared = _prep_shared(w_attn, b_attn, w_o, b_o, ln1_g, ln1_b, w_fc, b_fc,
                          w_pr, b_pr, ln2_g, ln2_b)

    in_maps = []
    idxs = []
    for c in range(N_CORES):
        b, j = c // 4, c % 4
        idx = np.r_[j * 256:(j + 1) * 256, (7 - j) * 256:(8 - j) * 256]
        idxs.append((b, idx))
        xT = np.ascontiguousarray(x[b, idx, :].T)  # [D, TOK]
        xs = xT.reshape(8, P, TOK).transpose(1, 0, 2)  # [P, 8, TOK]
        m = dict(shared)
        m["x_bf"] = np.ascontiguousarray(xs.astype(BF16))
        m["jidx"] = np.array([[j]], np.uint32)
        in_maps.append(m)

    res = run_bass_kernel_spmd(
        nc, in_maps, core_ids=list(range(N_CORES)), trace=_trace
    )
    if _trace:
        _CACHE["exec_time_ns"] = res.exec_time_ns
        it = getattr(res, "instructions_and_trace", None)
        _CACHE["trace_path"] = it[1] if it else None

    out = np.empty((B, S, D), np.float32)
    for c in range(N_CORES):
        b, idx = idxs[c]
        out[b, idx, :] = res.results[c]["out"].T
    return out


# revision 41
# speedup vs baseline: 1.2548x; 1.1099x over previous
"""Dense transformer block (attention + post-LN MLP) on 8 trn2 NeuronCores.

Context-parallel sharding: core c handles batch c//4 and token blocks
j*256, (7-j)*256 of that batch (j = c%4) -> causally balanced 512
tokens/core. Weights replicated (bf16).

v2 structure:
 - Host ships every tensor in exact SBUF layout (dense per-partition
   lines, one DMA each); the big phase-1 loads are issued first so the
   PE starts early.
 - K is projected FIRST and shipped as its own AllGather; V follows as
   two half-AllGathers (heads 0-7, 8-15); Q is projected while the
   rings run. Remote attention unblocks progressively.
 - Attention is two-pass: ALL head-pairs' local chunks run during the
   rings (partial osum evacuation), then the remote pass combines into
   osum. Deferred softmax normalization in two halves (hp 0-3 while
   hp 4-7 still compute) using reciprocal_approx_fast + GPSIMD
   partition_broadcast.
 - LayerNorms: bf16 cast + square fused into the producer phase,
   column-sum matmuls back-to-back, g-folded 2-op normalize, prewarmed
   Sqrt/Gelu tables. Residuals use bf16 x/n (tolerance 2e-2).
"""

import numpy as np
import ml_dtypes

BF16 = ml_dtypes.bfloat16

N_CORES = 8
B, S, D = 2, 2048, 1024
H, HD = 16, 64
F = 4 * D
TOK = 512            # tokens per core
P = 128
NHP = H // 2         # 8 head pairs
MASK_NEG = -80000.0  # -> -79872 in bf16; /8 => exp underflows to exactly 0
LN_EPS = 1e-5

VW = 66              # per-head V row width: 64 v + ones col + pad
KT_ELEMS = P * 8 * TOK           # K payload per core
V_ELEMS = P * 4 * (16 * VW)      # V payload per core

_CACHE = {}


def _q_blocks(j):
    """Global 128-blocks of this core's query tokens, ascending."""
    return [2 * j, 2 * j + 1, 14 - 2 * j, 15 - 2 * j]


def _remote_plan(j):
    """Remote chunks in assumed ring-arrival order (ranks j-1, j-2, j-3),
    each rank's valid chunks ascending.  Returns (mq, [(kc, slot, lt)],
    rank_of_slot)."""
    gq = _q_blocks(j)
    last_kc = gq[-1]
    plan = []
    rank_of_slot = []
    for d in range(1, 4):
        r = (j - d) % 4
        rb = _q_blocks(r)
        got = False
        for lt, kc in enumerate(rb):
            if kc <= last_kc and kc not in gq:
                plan.append((kc, len(rank_of_slot), lt))
                got = True
        if got:
            rank_of_slot.append(r)
    mq = 256 if j == 0 else 0
    return mq, plan, rank_of_slot


def _build():
    import concourse.bass as bass
    import concourse.mybir as mybir
    import concourse.tile as tile
    from concourse import bacc

    dt = mybir.dt
    AF = mybir.ActivationFunctionType
    OP = mybir.AluOpType

    nc = bacc.Bacc(
        "TRN2",
        target_bir_lowering=False,
        debug=False,
        enable_asserts=True,
        num_devices=N_CORES,
    )

    def din(name, shape, dty):
        return nc.dram_tensor(name, shape, dty, kind="ExternalInput").ap()

    # host ships everything in SBUF-exact layout
    x_bf = din("x_bf", [P, 8, TOK], dt.bfloat16)
    wq_d = din("wq_d", [P, 8, D], dt.bfloat16)     # Q columns of w_attn
    wk_d = din("wk_d", [P, 8, D], dt.bfloat16)     # K columns
    wv_d = din("wv_d", [P, 8, D], dt.bfloat16)     # V columns
    wo_d = din("wo_d", [P, 8, D], dt.bfloat16)
    wfc_d = din("wfc_d", [P, 8, F], dt.bfloat16)
    wpr_d = din("wpr_d", [P, 32, D], dt.bfloat16)
    # bias pack cols: bqk 16 | bo 8 | bfc 32 | bpr 8 | g1 8 | b1 8 | g2 8 | b2 8
    bias_d = din("bias_d", [P, 96], dt.float32)
    gcol_d = din("gcol_d", [1, 2 * D], dt.float32)  # g1 | g2 by feature
    im_d = din("im_d", [P, 2, P], dt.bfloat16)      # ident | maskm
    sel8_d = din("sel8", [8, 8 * 64], dt.bfloat16)  # one-hot row selectors
    jidx_d = din("jidx", [1, 1], dt.uint32)
    out_d = nc.dram_tensor("out", [D, TOK], dt.float32, kind="ExternalOutput").ap()

    with tile.TileContext(nc) as tc:
        from contextlib import ExitStack

        ctx = ExitStack()
        with ctx:
            c_pool = ctx.enter_context(tc.tile_pool(name="consts", bufs=1))
            dram = ctx.enter_context(tc.tile_pool(name="dram", bufs=1, space="DRAM"))

            # ---- long-lived pools (stack: pR > kvp > pA > gat > transient) ----
            pR_cm = tc.tile_pool(name="pR", bufs=1)
            pR = pR_cm.__enter__()
            kvp_cm = tc.tile_pool(name="kv_own", bufs=1)
            kvp = kvp_cm.__enter__()
            pA_cm = tc.tile_pool(name="pA", bufs=1)
            pA = pA_cm.__enter__()
            gat_cm = tc.tile_pool(name="gat", bufs=1)
            gat = gat_cm.__enter__()

            # ---- phase-1 critical loads FIRST (sync engine), split in halves
            # so the first matmuls can start before the full tiles land ----
            xb = pA.tile([P, 8, TOK], dt.bfloat16, tag="xb")
            xwk_cm = tc.tile_pool(name="xwk", bufs=2)
            xwk = xwk_cm.__enter__()
            wk = xwk.tile([P, 8, D], dt.bfloat16, tag="w8", name="wk")
            nc.sync.dma_start(xb[:, 0:4, :], x_bf[:, 0:4, :])
            nc.sync.dma_start(wk[:, 0:4, :], wk_d[:, 0:4, :])
            nc.sync.dma_start(xb[:, 4:8, :], x_bf[:, 4:8, :])
            nc.sync.dma_start(wk[:, 4:8, :], wk_d[:, 4:8, :])
            wv = xwk.tile([P, 8, D], dt.bfloat16, tag="w8", name="wv")
            nc.sync.dma_start(wv[:], wv_d[:])

            # ---- per-core j register (for Switch) ----
            jreg = nc.alloc_registers(
                "jreg",
                [mybir.EngineType.PE, mybir.EngineType.Activation,
                 mybir.EngineType.DVE, mybir.EngineType.SP,
                 mybir.EngineType.Pool],
            )
            nc.regs_load(jreg, jidx_d[0:1, 0:1])
            jval = nc.snap(jreg, donate=True, min_val=0, max_val=3)

            # ---- small consts (gpsimd queue keeps sync free) ----
            bias_sb = c_pool.tile([P, 96], dt.float32, tag="bias")
            nc.gpsimd.dma_start(bias_sb[:], bias_d[:])
            im_sb = c_pool.tile([P, 2, P], dt.bfloat16, tag="im")
            nc.gpsimd.dma_start(im_sb[:], im_d[:])
            ones128_bf = c_pool.tile([P, 1], dt.bfloat16, tag="ones128")
            nc.vector.memset(ones128_bf[:], 1.0)
            sel8 = c_pool.tile([8, 8, 64], dt.bfloat16, tag="sel8")
            nc.gpsimd.dma_start(
                sel8[:], sel8_d.rearrange("p (c f) -> p c f", f=64))
            sq_warm = c_pool.tile([1, 1], dt.float32, tag="sqwarm")
            nc.vector.memset(sq_warm[:], 1.0)

            bqk_sb = bias_sb[:, 0:16]
            bo_sb = bias_sb[:, 16:24]
            bfc_sb = bias_sb[:, 24:56]
            bpr_sb = bias_sb[:, 56:64]
            g1_sb = bias_sb[:, 64:72]
            b1_sb = bias_sb[:, 72:80]
            g2_sb = bias_sb[:, 80:88]
            b2_sb = bias_sb[:, 88:96]
            ident = im_sb[:, 0, :]
            maskm = im_sb[:, 1, :]

            # ---- AllGather buffers (fp8 payload, moved as uint8 bytes) ----
            agk_in = dram.tile([KT_ELEMS], dt.uint8, tag="agki", name="agki")
            agk_out = dram.tile([4, KT_ELEMS], dt.uint8, tag="agko", name="agko")
            agv_in = dram.tile([V_ELEMS], dt.uint8, tag="agvi", name="agvi")
            agv_out = dram.tile([4, V_ELEMS], dt.uint8, tag="agvo", name="agvo")
            GROUPS = [[0, 1, 2, 3], [4, 5, 6, 7]]

            # ---- long-lived SBUF tiles ----
            r1 = pR.tile([P, 8, TOK], dt.float32, tag="r1")

            v_own = kvp.tile([P, 4, 16 * VW], dt.bfloat16, tag="v_own")
            kT_own = kvp.tile([P, 8, TOK], dt.bfloat16, tag="kT_own")
            qT = kvp.tile([P, 8, TOK], dt.bfloat16, tag="qT")
            # fp8 copies: K/V for the collectives, Q for remote-chunk matmuls
            kT_f8 = kvp.tile([P, 8, TOK], dt.float8e4, tag="kT_f8")
            v_f8 = kvp.tile([P, 4, 16 * VW], dt.float8e4, tag="v_f8")
            qT_f8 = kvp.tile([P, 8, TOK], dt.float8e4, tag="qT_f8")

            o_cat = [pA.tile([P, TOK], dt.bfloat16, tag=f"o_cat{i}", name=f"o_cat{i}")
                     for i in range(NHP)]
            wo_sb = pA.tile([P, 8, D], dt.bfloat16, tag="wo")

            kT_s = [gat.tile([P, 8, TOK], dt.float8e4, tag=f"kTs{s}", name=f"kTs{s}")
                    for s in range(3)]
            v_s = [gat.tile([P, 4, 16 * VW], dt.float8e4, tag=f"vs{s}",
                            name=f"vs{s}") for s in range(3)]
            osum = [gat.tile([65, 2, TOK], dt.bfloat16, tag=f"osum{i}",
                             name=f"osum{i}") for i in range(NHP)]
            dcath = [gat.tile([4, 2, TOK], dt.bfloat16, tag=f"dcat{i}",
                              name=f"dcat{i}") for i in range(2)]
            dcat_f = gat.tile([4, 2, TOK], dt.float32, tag="dcat_f")
            rcat_f = gat.tile([4, 2, TOK], dt.float32, tag="rcat_f")
            rcath = [gat.tile([4, 2, TOK], dt.bfloat16, tag=f"rcat{i}",
                              name=f"rcat{i}") for i in range(2)]

            # ============ phase 1: K -> ship, V -> ship x2, Q ============
            with tc.tile_pool(name="qkv_ps", bufs=4, space="PSUM") as qkv_ps:

                def qk_quarter(w_sb, qi, dest, kbias, f8dest):
                    for fo in range(4):
                        fchunk = 4 * qi + fo
                        ps = qkv_ps.tile([P, TOK], dt.float32, tag="qkvps")
                        for c8 in range(8):
                            nc.tensor.matmul(
                                ps[:],
                                lhsT=w_sb[:, c8, qi * TOK + fo * P:
                                          qi * TOK + (fo + 1) * P],
                                rhs=xb[:, c8, :],
                                start=(c8 == 0),
                                stop=(c8 == 7),
                            )
                        bcol = fchunk + (8 if kbias else 0)
                        nc.vector.tensor_scalar(
                            dest[:, fchunk, :], ps[:],
                            bqk_sb[:, bcol:bcol + 1], None, OP.add,
                        )
                        nc.vector.tensor_copy(
                            f8dest[:, fchunk, :], dest[:, fchunk, :]
                        )

                # K first; ship its fp8 copy from the vector queue (the f8
                # copies precede it there, so deps resolve in FIFO order)
                qk_quarter(wk, 0, kT_own, True, kT_f8)
                qk_quarter(wk, 1, kT_own, True, kT_f8)
                nc.scalar.dma_start(
                    agk_in.rearrange("(p c t) -> p c t", p=P, c=8)
                    .bitcast(dt.float8e4),
                    kT_f8[:],
                )
                nc.gpsimd.collective_compute(
                    "AllGather", mybir.AluOpType.bypass,
                    replica_groups=GROUPS,
                    ins=[agk_in.opt()], outs=[agk_out.opt()],
                )
                # wq rotates into wk's slot once the K quarters are done
                wq = xwk.tile([P, 8, D], dt.bfloat16, tag="w8", name="wq")
                nc.sync.dma_start(wq[:], wq_d[:])

                # V next
                v4 = v_own.rearrange("p c (h w) -> p c h w", w=VW)
                v4_f8 = v_f8.rearrange("p c (h w) -> p c h w", w=VW)
                nc.vector.memset(v4[:, :, :, 64:66], 0.0)
                nc.vector.memset(v4[:, :, :, 64:65], 1.0)
                nc.vector.memset(v4_f8[:, :, :, 64:66], 0.0)
                nc.vector.memset(v4_f8[:, :, :, 64:65], 1.0)
                for t4 in range(4):
                    for vc in range(2):
                        ps = qkv_ps.tile([P, TOK], dt.float32, tag="qkvps")
                        for c8 in range(8):
                            nc.tensor.matmul(
                                ps[:],
                                lhsT=xb[:, c8, t4 * P:(t4 + 1) * P],
                                rhs=wv[:, c8, vc * TOK:(vc + 1) * TOK],
                                start=(c8 == 0),
                                stop=(c8 == 7),
                            )
                        dest = v4[:, t4, vc * 8:(vc + 1) * 8, 0:64]
                        nc.vector.tensor_copy(
                            dest, ps[:].rearrange("p (h w) -> p h w", w=64)
                        )
                        nc.vector.tensor_copy(
                            v4_f8[:, t4, vc * 8:(vc + 1) * 8, 0:64], dest
                        )
                nc.scalar.dma_start(
                    agv_in.rearrange("(p c f) -> p c f", p=P, c=4)
                    .bitcast(dt.float8e4),
                    v_f8[:],
                )
                nc.gpsimd.collective_compute(
                    "AllGather", mybir.AluOpType.bypass,
                    replica_groups=GROUPS,
                    ins=[agv_in.opt()], outs=[agv_out.opt()],
                )

                # Q last (overlaps the rings)
                qk_quarter(wq, 0, qT, False, qT_f8)
                qk_quarter(wq, 1, qT, False, qT_f8)

            xwk_cm.__exit__(None, None, None)

            # prefetch w_o for the post-attention projection
            nc.sync.dma_start(wo_sb[:], wo_d[:])

            # ============ attention ============
            with (
                tc.tile_pool(name="att_s", bufs=2, space="PSUM") as s_ps_pool,
                tc.tile_pool(name="att_o", bufs=2, space="PSUM") as o_ps_pool,
                tc.tile_pool(name="att_tmp", bufs=3) as atmp,
                tc.tile_pool(name="att_nrm", bufs=2) as anrm,
            ):
                def attn_qk_exp(hp, kc, gq, ksrc, f8):
                    """QK matmuls + exp; returns (aT, qs) for the deferred AV."""
                    vi = [i for i in range(4) if gq[i] >= kc]
                    qs = P * vi[0]
                    diag = gq[vi[0]] == kc
                    qsrc = qT_f8 if f8 else qT
                    sps = s_ps_pool.tile([P, 2, TOK], dt.float32, tag="sps")
                    for h2 in range(2):
                        lo, hi = 64 * h2, 64 * (h2 + 1)
                        nc.tensor.matmul(
                            sps[:, h2, qs:],
                            lhsT=ksrc[lo:hi, :],
                            rhs=qsrc[lo:hi, hp, qs:],
                            start=True,
                            stop=not diag,
                            tile_position=(64 * h2, 0),
                        )
                        if diag:
                            nc.tensor.matmul(
                                sps[:, h2, qs:qs + P],
                                lhsT=ident[:],
                                rhs=maskm[:],
                                start=False,
                                stop=True,
                            )
                    if f8:
                        aT = atmp.tile([P, 2, TOK], dt.float8e4, tag="aT8")
                    else:
                        aT = atmp.tile([P, 2, TOK], dt.bfloat16, tag="aT")
                    nc.scalar.activation(
                        aT[:, :, qs:], sps[:, :, qs:], AF.Exp, scale=0.125
                    )
                    return aT, qs

                def attn_av(hp, oT, aT, qs, vsrc, first, last):
                    for h2 in range(2):
                        h = 2 * hp + h2
                        nc.tensor.matmul(
                            oT[:, h2, qs:],
                            lhsT=vsrc[:, h * VW:h * VW + 65],
                            rhs=aT[:, h2, qs:],
                            start=first,
                            stop=last,
                        )

                def attn_pass(hp, oT, items, f8):
                    """items: [(kc, gq, ksrc, vsrc)].  AV issued one chunk
                    behind QK+exp so the exp stream saturates ACT."""
                    pend = None
                    n = len(items)
                    for ki, (kc, gq, ksrc, vsrc) in enumerate(items):
                        aT, qs = attn_qk_exp(hp, kc, gq, ksrc, f8)
                        if pend is not None:
                            attn_av(hp, oT, pend[0], pend[1], pend[2],
                                    ki == 1, False)
                        pend = (aT, qs, vsrc)
                    attn_av(hp, oT, pend[0], pend[1], pend[2], n == 1, True)

                def norm_half(h0):
                    """Deferred softmax normalization for hps h0..h0+3."""
                    half = h0 // 4
                    rc = rcath[half]
                    nc.vector.tensor_copy(dcat_f[:], dcath[half][:])
                    nc.vector.reciprocal_approx_fast(rcat_f[:], dcat_f[:])
                    nc.vector.tensor_copy(rc[:], rcat_f[:])
                    for hp in range(h0, h0 + 4):
                        rep = s_ps_pool.tile([P, 2, TOK], dt.float32, tag="sps")
                        for h2 in range(2):
                            nc.tensor.matmul(
                                rep[0:64, h2, :],
                                lhsT=sel8[0:4, hp - h0, :],
                                rhs=rc[:, h2, :],
                                start=True, stop=True,
                            )
                        ot = anrm.tile([64, TOK], dt.bfloat16, tag="otmp")
                        nc.vector.tensor_tensor(
                            o_cat[hp][0:64, :], osum[hp][0:64, 0, :],
                            rep[0:64, 0, :], OP.mult,
                        )
                        nc.vector.tensor_tensor(
                            ot[:], osum[hp][0:64, 1, :], rep[0:64, 1, :], OP.mult,
                        )
                        nc.sync.dma_start(o_cat[hp][64:128, :], ot[:])

                def attn_arm(j):
                    gq = _q_blocks(j)
                    mq, plan, rank_of_slot = _remote_plan(j)
                    # monotone dependency order on sync: K loads (ready
                    # first) then V loads -- no head-of-line blocking
                    for s, r in enumerate(rank_of_slot):
                        nc.sync.dma_start(
                            kT_s[s][:],
                            agk_out[r].rearrange("(p c t) -> p c t", p=P, c=8)
                            .bitcast(dt.float8e4),
                        )
                    for s, r in enumerate(rank_of_slot):
                        nc.sync.dma_start(
                            v_s[s][:],
                            agv_out[r].rearrange("(p c f) -> p c f", p=P, c=4)
                            .bitcast(dt.float8e4),
                        )
                    # pass 1: local chunks for ALL hps (overlaps the rings)
                    for hp in range(NHP):
                        oT = o_ps_pool.tile([65, 2, TOK], dt.float32, tag="oT")
                        attn_pass(hp, oT, [
                            (kc, gq, kT_own[:, hp, li * P:(li + 1) * P],
                             v_own[:, li, :])
                            for li, kc in enumerate(gq)], False)
                        nc.vector.tensor_copy(osum[hp][:], oT[:])
                    # pass 2: remote chunks, combine
                    for hp in range(NHP):
                        oT = o_ps_pool.tile([65, 2, TOK], dt.float32, tag="oT")
                        attn_pass(hp, oT, [
                            (kc, gq, kT_s[s][:, hp, lt * P:(lt + 1) * P],
                             v_s[s][:, lt, :])
                            for kc, s, lt in plan], True)
                        nc.vector.tensor_tensor(
                            osum[hp][:, :, mq:], osum[hp][:, :, mq:],
                            oT[:, :, mq:], OP.add,
                        )
                        nc.sync.dma_start(
                            dcath[hp // 4][hp % 4:hp % 4 + 1, :, :],
                            osum[hp][64:65, :, :])
                        if hp == 3:
                            norm_half(0)
                    norm_half(4)

                for case in tc.Switch(jval, 4):
                    attn_arm(case)

            gat_cm.__exit__(None, None, None)

            # prewarm Sqrt table for LN1 (ACT idle during w_o)
            nc.scalar.activation(sq_warm[:], sq_warm[:], AF.Sqrt)

            # ============ phase 4: w_o + residual -> r1 (+ LN1 prep) ============
            with tc.tile_pool(name="wo_ps", bufs=4, space="PSUM") as wo_ps:
                for oc in range(8):
                    ps = wo_ps.tile([P, TOK], dt.float32, tag="wops")
                    for hp in range(8):
                        nc.tensor.matmul(
                            ps[:],
                            lhsT=wo_sb[:, hp, oc * P:(oc + 1) * P],
                            rhs=o_cat[hp][:],
                            start=(hp == 0),
                            stop=(hp == 7),
                        )
                    nc.vector.scalar_tensor_tensor(
                        r1[:, oc, :], ps[:], bo_sb[:, oc:oc + 1], xb[:, oc, :],
                        op0=OP.add, op1=OP.add,
                    )

            pA_cm.__exit__(None, None, None)
            kvp_cm.__exit__(None, None, None)

            # ============ layernorm (feature-major, partition reduce) ============
            def layernorm(src, src_bf, sq, dst, g_sb, b_sb, gcol_off, tag):
                """dst[:, c8, :] = LN(src)[c8] * g + b.  src fp32 [P,8,TOK];
                src_bf/sq bf16 precomputed.  dst may alias src (per-c8 safe)."""
                with (
                    tc.tile_pool(name=f"ln_{tag}", bufs=1) as lnp,
                    tc.tile_pool(name=f"lnps_{tag}", bufs=1, space="PSUM") as lnps,
                    tc.tile_pool(name=f"lnsh_{tag}", bufs=2, space="PSUM") as lnsh,
                ):
                    s12 = lnps.tile([1, 2, TOK], dt.float32, tag=f"s12_{tag}")
                    for c8 in range(8):
                        nc.tensor.matmul(
                            s12[:, 0, :], lhsT=ones128_bf[:], rhs=src_bf[:, c8, :],
                            start=(c8 == 0), stop=(c8 == 7),
                        )
                        nc.tensor.matmul(
                            s12[:, 1, :], lhsT=ones128_bf[:], rhs=sq[:, c8, :],
                            start=(c8 == 0), stop=(c8 == 7),
                        )
                    stats = lnp.tile([1, 6, TOK], dt.float32, tag=f"st_{tag}")
                    mu = stats[:, 0, :]
                    m2 = stats[:, 1, :]
                    var = stats[:, 2, :]
                    rv = stats[:, 3, :]
                    rs = stats[:, 4, :]
                    musr = stats[:, 5, :]
                    nc.vector.tensor_scalar(mu, s12[:, 0, :], 1.0 / D, None, OP.mult)
                    nc.vector.tensor_scalar(m2, s12[:, 1, :], 1.0 / D, LN_EPS,
                                            OP.mult, OP.add)
                    nc.vector.tensor_tensor(var, mu, mu, OP.mult)
                    nc.vector.tensor_tensor(var, m2, var, OP.subtract)
                    nc.vector.reciprocal_approx_fast(rv, var)
                    nc.scalar.activation(rs, rv, AF.Sqrt)
                    nc.vector.tensor_tensor(musr, mu, rs, OP.mult)
                    rs_rep = lnp.tile([P, TOK], dt.float32, tag=f"rsr_{tag}")
                    nc.gpsimd.partition_broadcast(rs_rep[:], rs)
                    t = lnp.tile([P, TOK], dt.float32, tag=f"t_{tag}")
                    for c8 in range(8):
                        # shift = g_col(c8) (x) (mu * rs)   [P, TOK] in PSUM
                        sh = lnsh.tile([P, TOK], dt.float32, tag=f"sh_{tag}")
                        nc.tensor.matmul(
                            sh[:],
                            lhsT=gcol_sb[0:1, gcol_off + c8 * P:
                                         gcol_off + (c8 + 1) * P],
                            rhs=musr,
                            start=True, stop=True,
                        )
                        # t = (src * g_p) * rs_rep ; dst = (t + b_p) - shift
                        nc.vector.scalar_tensor_tensor(
                            t[:], src[:, c8, :], g_sb[:, c8:c8 + 1], rs_rep[:],
                            op0=OP.mult, op1=OP.mult,
                        )
                        nc.vector.scalar_tensor_tensor(
                            dst[:, c8, :], t[:], b_sb[:, c8:c8 + 1], sh[:],
                            op0=OP.add, op1=OP.subtract,
                        )

            pN_cm = tc.tile_pool(name="pN", bufs=1)
            pN = pN_cm.__enter__()
            n_bf = pN.tile([P, 8, TOK], dt.bfloat16, tag="n_bf")
            h1g = pN.tile([P, 32, TOK], dt.bfloat16, tag="h1g")
            r1bf = pN.tile([P, 8, TOK], dt.bfloat16, tag="r1bf")
            sq1 = pN.tile([P, 8, TOK], dt.bfloat16, tag="sq1")
            gcol_sb = pN.tile([1, 2 * D], dt.float32, tag="gcol")
            nc.gpsimd.dma_start(gcol_sb[:], gcol_d[:])
            for oc in range(8):
                nc.vector.tensor_copy(r1bf[:, oc, :], r1[:, oc, :])
                nc.gpsimd.tensor_tensor(
                    sq1[:, oc, :], r1[:, oc, :], r1[:, oc, :], OP.mult
                )

            # prefetch the first MLP weight quarters during LN1
            wprp_cm = tc.tile_pool(name="wpr", bufs=2)
            wprp = wprp_cm.__enter__()
            wpq0 = wprp.tile([P, 8, D], dt.bfloat16, tag="wprq", name="wpq0")
            nc.sync.dma_start(wpq0[:], wpr_d[:, 0:8, :])
            wfcp_cm = tc.tile_pool(name="wfc", bufs=2)
            wfcp = wfcp_cm.__enter__()
            wfq0 = wfcp.tile([P, 8, D], dt.bfloat16, tag="wfcq", name="wfq0")
            nc.sync.dma_start(wfq0[:], wfc_d[:, :, 0:D])

            layernorm(r1, r1bf, sq1, n_bf, g1_sb, b1_sb, 0, "ln1")

            # prewarm Gelu table while LN1's normalize runs on DVE
            nc.scalar.activation(sq_warm[:], sq_warm[:], AF.Gelu)

            # ============ phase 5: MLP ============
            with tc.tile_pool(name="fc_ps", bufs=4, space="PSUM") as fc_ps:
                for oq in range(4):
                    if oq == 0:
                        wfq = wfq0
                    else:
                        wfq = wfcp.tile([P, 8, D], dt.bfloat16, tag="wfcq")
                        nc.sync.dma_start(
                            wfq[:], wfc_d[:, :, oq * D:(oq + 1) * D])
                    for oc8 in range(8):
                        oc = oq * 8 + oc8
                        ps = fc_ps.tile([P, TOK], dt.float32, tag="fcps")
                        for c8 in range(8):
                            nc.tensor.matmul(
                                ps[:],
                                lhsT=wfq[:, c8, oc8 * P:(oc8 + 1) * P],
                                rhs=n_bf[:, c8, :],
                                start=(c8 == 0),
                                stop=(c8 == 7),
                            )
                        nc.scalar.activation(
                            h1g[:, oc, :], ps[:], AF.Gelu,
                            bias=bfc_sb[:, oc:oc + 1],
                        )

            wfcp_cm.__exit__(None, None, None)
            r2 = r1  # reuse r1's tile for the second residual
            r2bf = r1bf
            sq2 = sq1
            with tc.tile_pool(name="pr_ps", bufs=1, space="PSUM") as pr_ps:
                mps = [pr_ps.tile([P, TOK], dt.float32, tag=f"mps{i}",
                                  name=f"mps{i}") for i in range(8)]
                for q4 in range(4):
                    if q4 == 0:
                        wpq = wpq0
                    else:
                        wpq = wprp.tile([P, 8, D], dt.bfloat16, tag="wprq")
                        nc.sync.dma_start(
                            wpq[:], wpr_d[:, 8 * q4:8 * (q4 + 1), :])
                    for oc in range(8):
                        for c8 in range(8):
                            nc.tensor.matmul(
                                mps[oc][:],
                                lhsT=wpq[:, c8, oc * P:(oc + 1) * P],
                                rhs=h1g[:, q4 * 8 + c8, :],
                                start=(q4 == 0 and c8 == 0),
                                stop=(q4 == 3 and c8 == 7),
                            )
                # prewarm Sqrt for LN2 (ACT idle now)
                nc.scalar.activation(sq_warm[:], sq_warm[:], AF.Sqrt)
                for oc in range(8):
                    # r2 = mps + bpr + n   (n in bf16)
                    nc.vector.scalar_tensor_tensor(
                        r2[:, oc, :], mps[oc][:], bpr_sb[:, oc:oc + 1],
                        n_bf[:, oc, :], op0=OP.add, op1=OP.add,
                    )
                    nc.vector.tensor_copy(r2bf[:, oc, :], r2[:, oc, :])
                    nc.gpsimd.tensor_tensor(
                        sq2[:, oc, :], r2[:, oc, :], r2[:, oc, :], OP.mult
                    )
            wprp_cm.__exit__(None, None, None)

            layernorm(r2, r2bf, sq2, r2, g2_sb, b2_sb, D, "ln2")
            out_v = out_d.rearrange("(c p) t -> p c t", p=P)
            for c8 in range(8):
                nc.sync.dma_start(out_v[:, c8, :], r2[:, c8, :])
            pN_cm.__exit__(None, None, None)
            pR_cm.__exit__(None, None, None)

    nc.compile()
    return nc


def _prep_shared(w_attn, b_attn, w_o, b_o, ln1_g, ln1_b, w_fc, b_fc, w_pr, b_pr,
                 ln2_g, ln2_b):
    w_attn = np.asarray(w_attn, np.float32)
    b_attn = np.asarray(b_attn, np.float32)
    w_o_f = np.asarray(w_o, np.float32)
    b_v = b_attn[2 * D:]
    b_o_eff = (np.asarray(b_o, np.float32) + b_v @ w_o_f).astype(np.float32)
    mask = np.where(
        np.arange(P)[:, None] > np.arange(P)[None, :], MASK_NEG, 0.0
    ).astype(BF16)  # [ki, qj]: mask keys above the diagonal

    def sb(w):  # [D, Fo] -> [P, 8, Fo]  (feature-chunk-major SBUF layout)
        Fo = w.shape[1]
        return np.ascontiguousarray(
            w.reshape(8, P, Fo).transpose(1, 0, 2)).astype(BF16)

    bias_pack = np.zeros((P, 96), np.float32)
    bias_pack[:, 0:16] = b_attn[:2 * D].reshape(16, P).T
    bias_pack[:, 16:24] = b_o_eff.reshape(8, P).T
    bias_pack[:, 24:56] = np.asarray(b_fc, np.float32).reshape(32, P).T
    bias_pack[:, 56:64] = np.asarray(b_pr, np.float32).reshape(8, P).T
    bias_pack[:, 64:72] = np.asarray(ln1_g, np.float32).reshape(8, P).T
    bias_pack[:, 72:80] = np.asarray(ln1_b, np.float32).reshape(8, P).T
    bias_pack[:, 80:88] = np.asarray(ln2_g, np.float32).reshape(8, P).T
    bias_pack[:, 88:96] = np.asarray(ln2_b, np.float32).reshape(8, P).T

    gcol = np.concatenate([np.asarray(ln1_g, np.float32),
                           np.asarray(ln2_g, np.float32)])[None, :]

    im = np.stack([np.eye(P, dtype=np.float32).astype(BF16), mask], axis=1)

    wpr_f = np.asarray(w_pr, np.float32)  # [F, D]
    shared = {
        "wq_d": sb(w_attn[:, 0:D]),
        "wk_d": sb(w_attn[:, D:2 * D]),
        "wv_d": sb(np.ascontiguousarray(w_attn[:, 2 * D:])),
        "wo_d": sb(w_o_f),
        "wfc_d": sb(np.asarray(w_fc, np.float32)),
        "wpr_d": np.ascontiguousarray(
            wpr_f.reshape(32, P, D).transpose(1, 0, 2)).astype(BF16),
        "bias_d": bias_pack,
        "gcol_d": np.ascontiguousarray(gcol),
        "im_d": np.ascontiguousarray(im),
        "sel8": np.repeat(np.eye(8, dtype=np.float32), 64, axis=1).astype(BF16),
    }
    return shared


def kernel(x, w_attn, b_attn, w_o, b_o, ln1_g, ln1_b, w_fc, b_fc, w_pr, b_pr,
           ln2_g, ln2_b, _trace=False):
    from concourse.bass_utils import run_bass_kernel_spmd

    if "nc" not in _CACHE:
        _CACHE["nc"] = _build()
    nc = _CACHE["nc"]

    x = np.asarray(x, np.float32)
    shared = _prep_shared(w_attn, b_attn, w_o, b_o, ln1_g, ln1_b, w_fc, b_fc,
                          w_pr, b_pr, ln2_g, ln2_b)

    in_maps = []
    idxs = []
    for c in range(N_CORES):
        b, j = c // 4, c % 4
        idx = np.r_[j * 256:(j + 1) * 256, (7 - j) * 256:(8 - j) * 256]
        idxs.append((b, idx))
        xT = np.ascontiguousarray(x[b, idx, :].T)  # [D, TOK]
        xs = xT.reshape(8, P, TOK).transpose(1, 0, 2)  # [P, 8, TOK]
        m = dict(shared)
        m["x_bf"] = np.ascontiguousarray(xs.astype(BF16))
        m["jidx"] = np.array([[j]], np.uint32)
        in_maps.append(m)

    res = run_bass_kernel_spmd(
        nc, in_maps, core_ids=list(range(N_CORES)), trace=_trace
    )
    if _trace:
        _CACHE["exec_time_ns"] = res.exec_time_ns
        it = getattr(res, "instructions_and_trace", None)
        _CACHE["trace_path"] = it[1] if it else None

    out = np.empty((B, S, D), np.float32)
    for c in range(N_CORES):
        b, idx = idxs[c]
        out[b, idx, :] = res.results[c]["out"].T
    return out
